# revision 9
# baseline (speedup 1.0000x reference)
"""Modulated deformable conv (DCNv2) + eval-BN + ReLU on 8 TRN2 NeuronCores.

Sharding: 8 cores = (batch b in 0..3) x (image half h0 in {0, 48}).
Each core computes out[b, :, h0:h0+48, :] independently (no collectives).

Per-core algorithm (pos-major gather layout):
  - offset conv (3x3, 27ch) as 18 accumulating matmuls in c-major layout
  - PE-transpose offsets to pos-major packed (128pos, tile, 27)
  - field math on DVE/ACT -> bilinear tap weights (f32) + quad base index
  - gather: per (k, pos-tile) one indirect SWDGE DMA fetches a 2KB "quad row"
    per partition from a host-prepared xq table (4 taps x 256ch, bf16)
  - combine taps with per-partition tensor_scalar/ACT muls -> val (pos, k*c)
  - one batched SBUF->SBUF DMA-transpose per pos-tile -> val (ck, pos)
  - main conv: 36 accumulating matmuls per 4-tile group (N=512)
  - BN+ReLU via ACT with per-partition scale/bias, DMA out.

kernel(**inputs) takes the FULL unsharded inputs and returns the full
(4, 256, 96, 96) float32 output.
"""

import numpy as np
import ml_dtypes

import concourse.bass as bass
import concourse.tile as tile
import concourse.mybir as mybir
from concourse.bass_utils import run_bass_kernel_spmd

bf16 = mybir.dt.bfloat16
f32 = mybir.dt.float32
u32 = mybir.dt.uint32

K = 9
PAD = 6
H = 96
HP = H + 2 * PAD  # 108
NPIX = HP * HP  # 11664
NT = 41  # pos tiles of 128
LP = NT * 128  # 5248 >= 48*108
SLAB_ROWS = 53
SLAB = SLAB_ROWS * HP  # conv input slab: rows [h0+4, h0+57)
CONV_CHUNKS = [(i * 512, 512) for i in range(10)] + [(5120, 128)]
BN_EPS = 1e-5

_AF = mybir.ActivationFunctionType
_ALU = mybir.AluOpType


def _build_program():
    nc = bass.Bass()
    # ---- dram io ----
    xq_e = nc.dram_tensor("xq", [NPIX, 1024], bf16, kind="ExternalInput")
    xcm_e = nc.dram_tensor("xcm", [2, 128, SLAB], bf16, kind="ExternalInput")
    wofft_e = nc.dram_tensor("wofft", [128, 9 * 2 * 27], bf16, kind="ExternalInput")
    w2_e = nc.dram_tensor("w2", [128, 18 * 2 * 128], bf16, kind="ExternalInput")
    ident_e = nc.dram_tensor("ident", [128, 128], f32, kind="ExternalInput")
    basey_e = nc.dram_tensor("basey", [128, NT * K], f32, kind="ExternalInput")
    basex_e = nc.dram_tensor("basex", [128, NT * K], f32, kind="ExternalInput")
    basem_e = nc.dram_tensor("basem", [128, NT * K], f32, kind="ExternalInput")
    bnw_e = nc.dram_tensor("bnw", [128, 2], f32, kind="ExternalInput")
    bnb_e = nc.dram_tensor("bnb", [128, 2], f32, kind="ExternalInput")
    out_e = nc.dram_tensor("out", [256, LP], f32, kind="ExternalOutput")

    NK = NT * K  # 369

    with tile.TileContext(nc) as tc:
        with (
            tc.tile_pool(name="const", bufs=1) as cp,
            tc.tile_pool(name="field", bufs=1) as fp,
            tc.tile_pool(name="gpool", bufs=8) as gp,
            tc.tile_pool(name="tmp", bufs=8) as tp,
            tc.tile_pool(name="val", bufs=2) as vp,
            tc.tile_pool(name="valt", bufs=2) as vtp,
            tc.tile_pool(name="out_ps", bufs=2, space="PSUM") as outp,
            tc.tile_pool(name="osb", bufs=4) as osb_p,
        ):
            # ---- load constants ----
            xcm = [cp.tile([128, SLAB], bf16, name=f"xcm{c}", tag=f"xcm{c}") for c in range(2)]
            for c in range(2):
                nc.sync.dma_start(xcm[c][:], xcm_e[c])
            wofft = cp.tile([128, 9 * 2 * 27], bf16)
            nc.sync.dma_start(wofft[:], wofft_e[:])
            w2 = cp.tile([128, 18 * 2 * 128], bf16)
            nc.sync.dma_start(w2[:], w2_e[:])
            ident = cp.tile([128, 128], f32)
            nc.sync.dma_start(ident[:], ident_e[:])
            basey = cp.tile([128, NK], f32)
            nc.sync.dma_start(basey[:], basey_e[:])
            basex = cp.tile([128, NK], f32)
            nc.sync.dma_start(basex[:], basex_e[:])
            basem = cp.tile([128, NK], f32)
            nc.sync.dma_start(basem[:], basem_e[:])
            bnw = cp.tile([128, 2], f32)
            nc.sync.dma_start(bnw[:], bnw_e[:])
            bnb = cp.tile([128, 2], f32)
            nc.sync.dma_start(bnb[:], bnb_e[:])

            # ---- offset conv (c-major): off_cm (27, LP) f32 ----
            convtr = tc.tile_pool(name="conv_ps", bufs=2, space="PSUM")
            convp = convtr.__enter__()
            trctx = tc.tile_pool(name="tr_ps", bufs=2, space="PSUM")
            trp = trctx.__enter__()
            off_cm = fp.tile([32, LP], f32)
            taps = [(dy, dx) for dy in (-1, 0, 1) for dx in (-1, 0, 1)]
            for ci, (coff, clen) in enumerate(CONV_CHUNKS):
                ps = convp.tile([32, 512], f32, tag="convps")
                n = 0
                for ti, (dy, dx) in enumerate(taps):
                    for ch in range(2):
                        shift = 2 * HP + dy * HP + dx + coff  # pos0 at slab ofs 2*HP
                        nc.tensor.matmul(
                            ps[:27, :clen],
                            wofft[:, (ti * 2 + ch) * 27:(ti * 2 + ch) * 27 + 27],
                            xcm[ch][:, shift:shift + clen],
                            start=(n == 0),
                            stop=(n == 17),
                        )
                        n += 1
                nc.vector.tensor_copy(off_cm[:27, coff:coff + clen], ps[:27, :clen])

            # ---- transpose offsets to pos-major packed: offpk (128, NT*32) ----
            offpk = fp.tile([128, NT * 32], f32)
            for t in range(NT):
                pst = trp.tile([128, 32], f32, tag="trps")
                nc.tensor.transpose(
                    pst[:, :32], off_cm[:32, t * 128:(t + 1) * 128], ident[:32, :32]
                )
                nc.vector.tensor_copy(offpk[:, t * 32:(t + 1) * 32], pst[:])

            trctx.__exit__(None, None, None)
            convtr.__exit__(None, None, None)
            o3 = offpk[:].rearrange("p (t c) -> p t c", c=32)
            dy_all = o3[:, :, 0:18:2]   # (128, NT, 9)
            dx_all = o3[:, :, 1:18:2]
            ml_all = o3[:, :, 18:27]

            # ---- field math -> Wq (128, NK*4) f32, idx (128, NK) u32 ----
            pyt = fp.tile([128, NK], f32)
            pxt = fp.tile([128, NK], f32)
            fy = fp.tile([128, NK], f32)
            fx = fp.tile([128, NK], f32)
            y0 = fp.tile([128, NK], f32)
            x0 = fp.tile([128, NK], f32)
            msk = fp.tile([128, NK], f32)
            bb = fp.tile([128, NK], f32)
            aa = fp.tile([128, NK], f32)
            wx0 = fp.tile([128, NK], f32)
            idxf = fp.tile([128, NK], f32)
            idxu = fp.tile([128, NK], u32)
            wq = fp.tile([128, NK * 4], f32)
            w3 = wq[:].rearrange("p (n j) -> p n j", j=4)

            def v3(t128):  # view flat (128, NK) as (128, NT, 9)
                return t128[:].rearrange("p (t k) -> p t k", k=K)

            nc.vector.tensor_add(v3(pyt), dy_all, basey[:].rearrange("p (t k) -> p t k", k=K))
            nc.vector.tensor_add(v3(pxt), dx_all, basex[:].rearrange("p (t k) -> p t k", k=K))
            # floor: int-cast then correct for any rounding mode; exact fracs
            yi = fp.tile([128, NK], mybir.dt.int32)
            xi = fp.tile([128, NK], mybir.dt.int32)
            gt = fp.tile([128, NK], f32)
            nc.vector.tensor_copy(yi[:], pyt[:])
            nc.vector.tensor_copy(y0[:], yi[:])
            nc.vector.tensor_tensor(gt[:], y0[:], pyt[:], op=_ALU.is_gt)
            nc.vector.tensor_sub(y0[:], y0[:], gt[:])
            nc.vector.tensor_sub(fy[:], pyt[:], y0[:])
            nc.vector.tensor_copy(xi[:], pxt[:])
            nc.vector.tensor_copy(x0[:], xi[:])
            nc.vector.tensor_tensor(gt[:], x0[:], pxt[:], op=_ALU.is_gt)
            nc.vector.tensor_sub(x0[:], x0[:], gt[:])
            nc.vector.tensor_sub(fx[:], pxt[:], x0[:])
            # clamp to [0, HP-2]
            nc.vector.tensor_scalar(y0[:], y0[:], 0.0, float(HP - 2), op0=_ALU.max, op1=_ALU.min)
            nc.vector.tensor_scalar(x0[:], x0[:], 0.0, float(HP - 2), op0=_ALU.max, op1=_ALU.min)
            # quad index = y0*HP + x0
            nc.vector.tensor_scalar(idxf[:], y0[:], float(HP), None, op0=_ALU.mult)
            nc.vector.tensor_add(idxf[:], idxf[:], x0[:])
            nc.vector.tensor_copy(idxu[:], idxf[:])
            # mask = sigmoid(mlogit + basem)
            nc.vector.tensor_add(v3(msk), ml_all, basem[:].rearrange("p (t k) -> p t k", k=K))
            nc.scalar.activation(msk[:], msk[:], _AF.Sigmoid)
            # tap weights
            nc.vector.tensor_mul(bb[:], msk[:], fy[:])          # b = m*fy
            nc.vector.tensor_sub(aa[:], msk[:], bb[:])          # a = m*(1-fy)
            nc.vector.tensor_scalar(wx0[:], fx[:], -1.0, 1.0, op0=_ALU.mult, op1=_ALU.add)
            nc.vector.tensor_mul(w3[:, :, 0], aa[:], wx0[:])    # w00  (y0,x0)
            nc.vector.tensor_mul(w3[:, :, 1], bb[:], wx0[:])    # w10  (y1,x0)
            nc.vector.tensor_mul(w3[:, :, 2], aa[:], fx[:])     # w01  (y0,x1)
            nc.vector.tensor_mul(w3[:, :, 3], bb[:], fx[:])     # w11  (y1,x1)

            # ---- main loop ----
            ngroup = (NT + 3) // 4
            for g in range(ngroup):
                tlo = g * 4
                tn = min(4, NT - tlo)
                valt = vtp.tile([128, 18 * 512], bf16, tag="valt")
                vt3 = valt[:].rearrange("p (j n) -> p j n", n=512)
                for tt in range(tn):
                    t = tlo + tt
                    val = vp.tile([128, 2304], bf16, tag="val")
                    for kk in range(K):
                        slot = t * K + kk
                        g_t = gp.tile([128, 1024], bf16, tag="g")
                        nc.gpsimd.indirect_dma_start(
                            out=g_t[:],
                            out_offset=None,
                            in_=xq_e[:],
                            in_offset=bass.IndirectOffsetOnAxis(
                                ap=idxu[:, slot:slot + 1], axis=0
                            ),
                        )
                        # quad layout: [v00, v10, v01, v11] x 256ch
                        m0 = tp.tile([128, 256], bf16, tag="m0")
                        m1 = tp.tile([128, 256], bf16, tag="m1")
                        m3 = tp.tile([128, 256], bf16, tag="m3")
                        vs = val[:, kk * 256:(kk + 1) * 256]
                        nc.vector.tensor_scalar(
                            m0[:], g_t[:, 0:256], wq[:, slot * 4:slot * 4 + 1],
                            None, op0=_ALU.mult
                        )
                        nc.scalar.activation(
                            m1[:], g_t[:, 256:512], _AF.Copy,
                            scale=wq[:, slot * 4 + 1:slot * 4 + 2]
                        )
                        nc.vector.tensor_scalar(
                            vs, g_t[:, 512:768], wq[:, slot * 4 + 2:slot * 4 + 3],
                            None, op0=_ALU.mult
                        )
                        nc.scalar.activation(
                            m3[:], g_t[:, 768:1024], _AF.Copy,
                            scale=wq[:, slot * 4 + 3:slot * 4 + 4]
                        )
                        nc.vector.tensor_add(m0[:], m0[:], m1[:])
                        nc.vector.tensor_add(m0[:], m0[:], m3[:])
                        nc.vector.tensor_add(vs, vs, m0[:])
                    nc.sync.dma_start_transpose(
                        vt3[:, :, tt * 128:(tt + 1) * 128], val[:]
                    )
                # matmuls for the group
                pso = [outp.tile([128, 512], f32, name=f"pso{oh}", tag=f"ops{oh}") for oh in range(2)]
                for oh in range(2):
                    for j in range(18):
                        nc.tensor.matmul(
                            pso[oh][:, :tn * 128],
                            w2[:, (j * 2 + oh) * 128:(j * 2 + oh) * 128 + 128],
                            valt[:, j * 512:j * 512 + tn * 128],
                            start=(j == 0),
                            stop=(j == 17),
                        )
                    ob = osb_p.tile([128, 512], f32, tag="ob")
                    nc.scalar.activation(
                        ob[:, :tn * 128], pso[oh][:, :tn * 128], _AF.Relu,
                        bias=bnb[:, oh:oh + 1], scale=bnw[:, oh:oh + 1],
                    )
                    nc.sync.dma_start(
                        out_e[oh * 128:(oh + 1) * 128, tlo * 128:tlo * 128 + tn * 128],
                        ob[:, :tn * 128],
                    )
    _split_multi_waits(nc)
    return nc


def _split_multi_waits(nc, maxw=1):
    """The walrus build here rejects instructions with >1 semaphore wait
    ("Too many sync wait commands"); hoist excess waits onto standalone
    event-semaphore instructions right before the offender (same engine
    stream => semantics preserved)."""
    n_fixed = 0
    for fn in nc.m.functions:
        for blk in fn.blocks:
            il = blk.instructions
            i = 0
            while i < len(il):
                inst = il[i]
                si = inst.sync_info
                if si is not None and len(si.on_wait) > maxw:
                    waits = list(si.on_wait)
                    keep = waits[:maxw - 1] if maxw > 1 else []
                    hoist = waits[len(keep):-1] if maxw > 1 else waits[:-1]
                    inst.sync_info = mybir.SyncInfo(
                        on_wait=keep + [waits[-1]], on_update=list(si.on_update)
                    )
                    for j, w in enumerate(hoist):
                        ev = mybir.InstEventSemaphore(
                            name=f"{inst.name}-hw{j}", ins=[], outs=[]
                        )
                        ev.engine = inst.engine
                        ev.sync_info = mybir.SyncInfo(on_wait=[w], on_update=[])
                        il.insert(i, ev)
                        i += 1
                    n_fixed += 1
                i += 1
    return n_fixed


# ---------------- host side ----------------

def _prep_inputs(input_x, w_off, b_off, w, b, gamma, beta, rmean, rvar):
    B = input_x.shape[0]
    x = np.asarray(input_x, np.float32)
    xbf = x.astype(ml_dtypes.bfloat16)
    # padded image per batch, bf16 values
    xp = np.zeros((B, 256, HP, HP), ml_dtypes.bfloat16)
    xp[:, :, PAD:PAD + H, PAD:PAD + H] = xbf
    # xq: (B, NPIX, 4*256) quad rows
    xpp = np.zeros((B, 256, HP + 1, HP + 1), ml_dtypes.bfloat16)
    xpp[:, :, :HP, :HP] = xp
    ys, xs = np.divmod(np.arange(NPIX), HP)
    xq = np.empty((B, NPIX, 4, 256), ml_dtypes.bfloat16)
    for j, (dy, dx) in enumerate(((0, 0), (1, 0), (0, 1), (1, 1))):
        xq[:, :, j, :] = xpp[:, :, ys + dy, xs + dx].transpose(0, 2, 1)
    xq = xq.reshape(B, NPIX, 1024)

    wofft = np.empty((128, 9, 2, 27), ml_dtypes.bfloat16)
    wf = np.asarray(w_off, np.float32)  # (27, 256, 3, 3)
    for ti in range(9):
        ty, tx = divmod(ti, 3)
        for ch in range(2):
            wofft[:, ti, ch, :] = wf[:, ch * 128:(ch + 1) * 128, ty, tx].T.astype(
                ml_dtypes.bfloat16)
    wofft = wofft.reshape(128, 9 * 2 * 27)

    wr = np.asarray(w, np.float32).reshape(256, 256, 9)  # (O, C, K)
    w2 = np.empty((128, 18, 2, 128), ml_dtypes.bfloat16)
    for kk in range(9):
        for ch in range(2):
            j = 2 * kk + ch
            for oh in range(2):
                # lhsT[cc, oo] = w[oh*128+oo, ch*128+cc, kk]
                w2[:, j, oh, :] = wr[oh * 128:(oh + 1) * 128,
                                     ch * 128:(ch + 1) * 128, kk].T.astype(
                    ml_dtypes.bfloat16)
    w2 = w2.reshape(128, 18 * 2 * 128)

    ident = np.eye(128, dtype=np.float32)

    scale = (np.asarray(gamma, np.float32)
             / np.sqrt(np.asarray(rvar, np.float32) + BN_EPS))
    bias_tot = (np.asarray(b, np.float32) * scale
                + np.asarray(beta, np.float32)
                - np.asarray(rmean, np.float32) * scale)
    bnw = scale.reshape(2, 128).T.copy()  # (128, 2)
    bnb = bias_tot.reshape(2, 128).T.copy()

    ky = (np.arange(K) // 3 - 1).astype(np.float32)
    kx = (np.arange(K) % 3 - 1).astype(np.float32)
    boff = np.asarray(b_off, np.float32)

    per_core = []
    for core in range(8):
        bidx, half = divmod(core, 2)
        h0 = half * 48
        s0 = (h0 + PAD) * HP
        s = s0 + (np.arange(NT)[None, :, None] * 128
                  + np.arange(128)[:, None, None])  # (128, NT, 1)
        ypad, xpad = np.divmod(s, HP)
        basey = (ypad + ky[None, None, :] + boff[0:18:2][None, None, :]).astype(np.float32)
        basex = (xpad + kx[None, None, :] + boff[1:18:2][None, None, :]).astype(np.float32)
        basem = np.broadcast_to(boff[18:27][None, None, :], basey.shape).astype(np.float32)
        # conv slab rows [h0+5, h0+57)
        slab = np.ascontiguousarray(
            xp[bidx, :, h0 + 4:h0 + 4 + SLAB_ROWS, :].reshape(256, SLAB)
            .reshape(2, 128, SLAB))
        per_core.append({
            "xq": np.ascontiguousarray(xq[bidx]),
            "xcm": slab,
            "wofft": wofft,
            "w2": w2,
            "ident": ident,
            "basey": np.ascontiguousarray(basey.reshape(128, NT * K)),
            "basex": np.ascontiguousarray(basex.reshape(128, NT * K)),
            "basem": np.ascontiguousarray(basem.reshape(128, NT * K)),
            "bnw": np.ascontiguousarray(bnw),
            "bnb": np.ascontiguousarray(bnb),
        })
    return per_core


_PROG_CACHE = {}


def _get_program():
    if "nc" not in _PROG_CACHE:
        _PROG_CACHE["nc"] = _build_program()
    return _PROG_CACHE["nc"]


def kernel(**inputs, ):
    return _run(inputs, trace=False)[0]


def _run(inputs, trace=False):
    per_core = _prep_inputs(**inputs)
    nc = _get_program()
    res = run_bass_kernel_spmd(nc, per_core, list(range(8)), trace=trace)
    out = np.empty((4, 256, 96, 96), np.float32)
    for core in range(8):
        bidx, half = divmod(core, 2)
        h0 = half * 48
        slab = res.results[core]["out"][:, :48 * HP].reshape(256, 48, HP)
        out[bidx, :, h0:h0 + 48, :] = slab[:, :, PAD:PAD + H]
    return out, res.exec_time_ns
